# revision 6
# baseline (speedup 1.0000x reference)
"""Delayed synaptic layer on 8 Trainium2 NeuronCores.

Math: out[b,q] = sum_p weight[p,q] * interp(buf[b,:,p], d[p,q]),
      d = 50*sigmoid(delay_raw), interp = linear interpolation over t.

Identity (exact): with clip01(x) = min(max(x,0),1),
  out = buf[:,0,:] @ W + sum_{j=0}^{49} g_j @ (W * clip01(d-j)),  g_j = buf[:,j+1]-buf[:,j]

The per-step clip+mult chain is split across TWO engines by output column
so neither is the lone bottleneck (ScalarE relu was the old 3.6us/step
bottleneck; DVE clamp+mult alone would be 3.3us/step):

 - DVE columns (168/256 per pt): y = tensor_scalar 2-op clamp(dB, i-4, i-3)
   at 4x (fp16 single-src), where dB = 50*sigmoid(delay) - (8*B+4) is a
   per-8-step-block recentered copy (keeps |y|<=4 so the fp16 product
   w*y stays accurate). y = clip01(d-j) + (i-4); the accumulated offset
   sum_j (j%8-4)*g_j telescopes to L = S_all - 4*buf0 + 2*buf50 - 8*S_8
   (S_all = sum_t buf_t, S_8 = sum_{k=1..6} buf_{8k}), applied as two
   fp16 (hi+lo) correction matmuls at the end. S_all/S_8 reduce on the
   otherwise-idle GPSIMD engine during the loop.
 - ScalarE columns (88/256 per pt): exact relu countdown chain
   u_{j+1} = relu(u_j - 1) (integer steps: exact in fp16), v_j = relu(1-u_j)
   = 1 - clip01(d-j). Sign is folded into host-negated w columns; the
   flip constant buf50 @ w is one extra correction matmul set.

One full-width DVE tensor_tensor mult (cv*w, fp16 2x) + 16 col-strip
packed matmuls per step as before. Steady state ~= max(DVE 2954ns,
ScalarE 2933ns, PE ~2.1us) per step.

Sharding: columns (n_post) split across the 8 cores; buf replicated; host
does layout/dtype prep + column sign flips only; all arithmetic on-device.
"""

import numpy as np

B, T, P, QFULL = 16, 51, 2048, 2048
NCORES = 8
Q = QFULL // NCORES          # 256 output columns per core
NPT = P // 128               # 16 partition tiles over pre-neurons
NS = T - 1                   # 50 clip terms
FD = NPT * Q                 # 4096 free-dim elements per [128, .] pass
SCE = 88                     # ScalarE-chain columns per 256-col pt block
DVE = Q - SCE                # DVE-clamp columns per pt block
NBLK = (NS + 7) // 8         # 7 blocks of 8 steps

_CACHE = {}


def _build_program():
    import concourse.bass as bass
    import concourse.mybir as mybir
    from concourse.tile import TileContext

    fp32 = mybir.dt.float32
    fp16 = mybir.dt.float16
    Act = mybir.ActivationFunctionType
    Alu = mybir.AluOpType

    nc = bass.Bass()
    buft_d = nc.dram_tensor("buft", [128, NPT * T * B], fp16, kind="ExternalInput")
    w_d = nc.dram_tensor("w", [128, FD], fp16, kind="ExternalInput")
    delay_d = nc.dram_tensor("delay", [128, FD], fp32, kind="ExternalInput")
    out_d = nc.dram_tensor("out", [B, Q], fp32, kind="ExternalOutput")

    with TileContext(nc) as tc:
        with (
            tc.tile_pool(name="persist", bufs=1) as persist,
            tc.tile_pool(name="dpool", bufs=2) as dpool,
            tc.tile_pool(name="upool", bufs=2) as upool,
            tc.tile_pool(name="cvpool", bufs=3) as cvpool,
            tc.tile_pool(name="rpool", bufs=3) as rpool,
            tc.tile_pool(name="psump", bufs=1, space="PSUM") as psump,
        ):
            buft = persist.tile([128, NPT * T * B], fp16, tag="buft")
            w = persist.tile([128, FD], fp16, tag="w")
            delay = persist.tile([128, FD], fp32, tag="delay")
            sg = persist.tile([128, FD], fp32, tag="sg")
            d50mid = persist.tile([128, FD], fp16, tag="d50mid")
            # delay first: sigmoid -> d50mid -> first block tile is the
            # critical path into the loop; chunk 4-way so it starts early.
            H = FD // 4
            for h in range(4):
                sl = slice(h * H, (h + 1) * H)
                nc.sync.dma_start(out=delay[:, sl], in_=delay_d[:, sl])
            nc.sync.dma_start(out=buft[:], in_=buft_d[:])
            nc.sync.dma_start(out=w[:], in_=w_d[:])
            for h in range(4):
                sl = slice(h * H, (h + 1) * H)
                nc.scalar.activation(sg[:, sl], delay[:, sl], Act.Sigmoid)
                # d50mid = 50*sg - 25, centered master copy (fp16)
                nc.vector.tensor_scalar(
                    d50mid[:, sl], sg[:, sl], 50.0, 25.0, Alu.mult, Alu.subtract
                )

            buft_v = buft[:].rearrange("p (pt t b) -> p pt t b", pt=NPT, t=T, b=B)
            # gT[pr, pt, s, b] = buf[b, s+1, p] - buf[b, s, p]
            gT = persist.tile([128, NPT * NS * B], fp16, tag="gT")
            gT_v = gT[:].rearrange("p (pt s b) -> p pt s b", pt=NPT, s=NS, b=B)
            nc.vector.tensor_tensor(
                gT_v, buft_v[:, :, 1:, :], buft_v[:, :, : NS, :], Alu.subtract
            )

            # Reductions for the offset correction (emitted here; Tile
            # schedules by dependency so they fill DVE slack around the loop).
            # S_all[p, pt, b] = sum_t buf; S_8 = sum over t in {8,..,48}
            s_all = persist.tile([128, NPT * B], fp32, tag="s_all")
            s8 = persist.tile([128, NPT * B], fp32, tag="s8")
            s_all_v = s_all[:].rearrange("p (pt b) -> p pt b", pt=NPT, b=B)
            s8_v = s8[:].rearrange("p (pt b) -> p pt b", pt=NPT, b=B)
            buft_tinner = buft[:].rearrange(
                "p (pt t b) -> p pt b t", pt=NPT, t=T, b=B
            )
            nc.vector.reduce_sum(s_all_v, buft_tinner, mybir.AxisListType.X)
            nc.vector.reduce_sum(s8_v, buft_tinner[:, :, :, 8:49:8], mybir.AxisListType.X)

            # absorb the w DMA-completion wait off the loop's first multiply
            wtouch = persist.tile([128, 2], fp16, tag="wtouch")
            nc.vector.tensor_copy(wtouch[:], w[:, 0:2])

            # per-partition bias columns for the ScalarE relu chain
            biases = persist.tile([128, 4], fp32, tag="biases")
            b_m24, b_p24 = biases[:, 0:1], biases[:, 1:2]
            b_p1, b_m1 = biases[:, 2:3], biases[:, 3:4]
            nc.gpsimd.memset(b_m24, -24.0)
            nc.gpsimd.memset(b_p24, 24.0)
            nc.gpsimd.memset(b_p1, 1.0)
            nc.gpsimd.memset(b_m1, -1.0)

            psum = psump.tile([128, Q], fp32, tag="acc")

            d50mid_v = d50mid[:].rearrange("p (pt q) -> p pt q", pt=NPT, q=Q)
            w_v = w[:].rearrange("p (pt q) -> p pt q", pt=NPT, q=Q)

            ucur = None
            for j in range(NS):
                blk, i = j >> 3, j & 7
                if i == 0:
                    # per-block recentered tile, DVE columns only:
                    # dB = d50mid - (8*blk - 21)  (= d - (8*blk + 4))
                    dtile = dpool.tile([128, FD], fp16, tag="dtile")
                    dtile_v = dtile[:].rearrange("p (pt q) -> p pt q", pt=NPT, q=Q)
                    nc.vector.tensor_scalar(
                        dtile_v[:, :, SCE:],
                        d50mid_v[:, :, SCE:],
                        float(8 * blk - 21),
                        None,
                        Alu.subtract,
                    )
                cv = cvpool.tile([128, FD], fp16, tag="cv")
                cv_v = cv[:].rearrange("p (pt q) -> p pt q", pt=NPT, q=Q)
                # DVE cols: y = clamp(dB, i-4, i-3) = clip01(d-j) + (i-4)
                nc.vector.tensor_scalar(
                    cv_v[:, :, SCE:],
                    dtile_v[:, :, SCE:],
                    float(i - 4),
                    float(i - 3),
                    Alu.max,
                    Alu.min,
                )
                # ScalarE cols: v = relu(1 - u_j) = 1 - clip01(d-j);
                # u chain: u_{j+1} = relu(u_j - 1), u_0 = d = d50mid + 25
                if j == 0:
                    nc.scalar.activation(
                        cv_v[:, :, :SCE], d50mid_v[:, :, :SCE], Act.Relu,
                        bias=b_m24, scale=-1.0,
                    )
                    unext = upool.tile([128, NPT * SCE], fp16, tag="u")
                    unext_v = unext[:].rearrange("p (pt q) -> p pt q", pt=NPT, q=SCE)
                    nc.scalar.activation(
                        unext_v, d50mid_v[:, :, :SCE], Act.Relu, bias=b_p24
                    )
                    ucur = unext
                else:
                    ucur_v = ucur[:].rearrange("p (pt q) -> p pt q", pt=NPT, q=SCE)
                    nc.scalar.activation(
                        cv_v[:, :, :SCE], ucur_v, Act.Relu, bias=b_p1, scale=-1.0
                    )
                    if j < NS - 1:
                        unext = upool.tile([128, NPT * SCE], fp16, tag="u")
                        nc.scalar.activation(unext[:], ucur[:], Act.Relu, bias=b_m1)
                        ucur = unext

                r = rpool.tile([128, FD], fp16, tag="rhs")
                nc.vector.tensor_tensor(r[:], cv[:], w[:], Alu.mult)
                for pt in range(NPT):
                    strip = pt % 4
                    nc.tensor.matmul(
                        psum[32 * strip : 32 * strip + B, :],
                        lhsT=gT_v[:, pt, j, :],
                        rhs=r[:, pt * Q : (pt + 1) * Q],
                        start=(j == 0 and pt < 4),
                        stop=False,
                        tile_position=(0, 32 * strip),
                        skip_group_check=True,
                    )

            # ---- corrections ----
            # L = S_all - 8*S_8 - 4*buf0 + 2*buf50   (fp32, then hi+lo fp16)
            t8 = persist.tile([128, NPT * B], fp32, tag="t8")
            nc.vector.tensor_scalar(t8[:], s8[:], 8.0, None, Alu.mult)
            L = persist.tile([128, NPT * B], fp32, tag="L")
            nc.vector.tensor_tensor(L[:], s_all[:], t8[:], Alu.subtract)
            t0 = persist.tile([128, NPT * B], fp32, tag="t0")
            t0_v = t0[:].rearrange("p (pt b) -> p pt b", pt=NPT, b=B)
            nc.vector.tensor_scalar(t0_v, buft_v[:, :, 0, :], 4.0, None, Alu.mult)
            nc.vector.tensor_tensor(L[:], L[:], t0[:], Alu.subtract)
            t50 = persist.tile([128, NPT * B], fp32, tag="t50")
            t50_v = t50[:].rearrange("p (pt b) -> p pt b", pt=NPT, b=B)
            nc.vector.tensor_scalar(t50_v, buft_v[:, :, 50, :], 2.0, None, Alu.mult)
            nc.vector.tensor_tensor(L[:], L[:], t50[:], Alu.add)
            L_hi = persist.tile([128, NPT * B], fp16, tag="L_hi")
            nc.vector.tensor_copy(L_hi[:], L[:])
            L_lo = persist.tile([128, NPT * B], fp16, tag="L_lo")
            nc.vector.tensor_tensor(L_lo[:], L[:], L_hi[:], Alu.subtract)
            mb50 = persist.tile([128, NPT * B], fp16, tag="mb50")
            mb50_v = mb50[:].rearrange("p (pt b) -> p pt b", pt=NPT, b=B)
            nc.vector.tensor_scalar(mb50_v, buft_v[:, :, 50, :], -1.0, None, Alu.mult)

            L_hi_v = L_hi[:].rearrange("p (pt b) -> p pt b", pt=NPT, b=B)
            L_lo_v = L_lo[:].rearrange("p (pt b) -> p pt b", pt=NPT, b=B)
            for pt in range(NPT):
                strip = pt % 4
                last = pt >= NPT - 4
                rows = psum[32 * strip : 32 * strip + B, :]
                # ScalarE cols: += (-buf50) @ (-w) = buf50 @ w
                nc.tensor.matmul(
                    rows[:, 0:SCE],
                    lhsT=mb50_v[:, pt, :],
                    rhs=w_v[:, pt, 0:SCE],
                    start=False, stop=last,
                    tile_position=(0, 32 * strip),
                    skip_group_check=True,
                )
                # DVE cols: += (L_hi + L_lo) @ w
                nc.tensor.matmul(
                    rows[:, SCE:],
                    lhsT=L_hi_v[:, pt, :],
                    rhs=w_v[:, pt, SCE:],
                    start=False, stop=False,
                    tile_position=(0, 32 * strip),
                    skip_group_check=True,
                )
                nc.tensor.matmul(
                    rows[:, SCE:],
                    lhsT=L_lo_v[:, pt, :],
                    rhs=w_v[:, pt, SCE:],
                    start=False, stop=last,
                    tile_position=(0, 32 * strip),
                    skip_group_check=True,
                )

            out_sb = persist.tile([B, Q], fp32, tag="out_sb")
            nc.scalar.copy(out_sb[:], psum[0:B, :])
            for strip in range(1, 4):
                nc.vector.tensor_tensor(
                    out_sb[:], out_sb[:], psum[32 * strip : 32 * strip + B, :], Alu.add
                )
            nc.sync.dma_start(out=out_d[:], in_=out_sb[:])

    return nc


def _split_multi_waits(nc):
    """Walrus encodes at most one sync-wait per 64B instruction for several
    TRN2 instruction formats; Tile can attach two. Move excess waits onto
    injected same-engine NoOp carriers placed immediately before."""
    import concourse.mybir as mybir

    for fn in nc.m.functions:
        for bb in fn.blocks:
            il = bb.instructions
            out = []
            changed = False
            for ins in il:
                si = ins.sync_info
                if si is not None and si.on_wait and len(si.on_wait) > 1:
                    waits = list(si.on_wait)
                    for w in waits[:-1]:
                        out.append(
                            mybir.InstNoOp(
                                name=nc.get_next_instruction_name(),
                                engine=ins.engine,
                                ins=[],
                                outs=[],
                                sync_info=mybir.SyncInfo(on_wait=[w], on_update=[]),
                            )
                        )
                    ins.sync_info = mybir.SyncInfo(
                        on_wait=[waits[-1]], on_update=list(si.on_update or [])
                    )
                    changed = True
                out.append(ins)
            if changed:
                il[:] = out


def _get_program(split_waits=True):
    key = ("nc", split_waits)
    if key not in _CACHE:
        nc = _build_program()
        if split_waits:
            _split_multi_waits(nc)
        _CACHE[key] = nc
    return _CACHE[key]


def _host_layouts(buf, weight, delay_raw):
    # bufT[pr, pt, t, b] = buf[b, t, pt*128+pr], flattened to [128, NPT*T*B]
    bufT = (
        np.ascontiguousarray(
            buf.transpose(2, 1, 0)  # [P, T, B]
            .reshape(NPT, 128, T, B)
            .transpose(1, 0, 2, 3)  # [128, NPT, T, B]
        )
        .reshape(128, NPT * T * B)
        .astype(np.float16)
    )
    # per-core column slices, [128, NPT, Q] -> [128, FD]; the first SCE
    # columns of each pt block carry negated weights (ScalarE chain path).
    ws, ds = [], []
    for c in range(NCORES):
        wq = weight[:, c * Q : (c + 1) * Q].reshape(NPT, 128, Q).transpose(1, 0, 2)
        wq = wq.copy()
        wq[:, :, :SCE] = -wq[:, :, :SCE]
        dq = delay_raw[:, c * Q : (c + 1) * Q].reshape(NPT, 128, Q).transpose(1, 0, 2)
        ws.append(np.ascontiguousarray(wq).reshape(128, FD).astype(np.float16))
        ds.append(np.ascontiguousarray(dq).reshape(128, FD).astype(np.float32))
    return bufT, ws, ds


def kernel(buf, weight, delay_raw):
    from concourse.bass_utils import run_bass_kernel_spmd

    buf = np.asarray(buf, dtype=np.float32)
    weight = np.asarray(weight, dtype=np.float32)
    delay_raw = np.asarray(delay_raw, dtype=np.float32)

    nc = _get_program()
    bufT, ws, ds = _host_layouts(buf, weight, delay_raw)
    in_maps = [
        {"buft": bufT, "w": ws[c], "delay": ds[c]} for c in range(NCORES)
    ]
    last_err = None
    for _attempt in range(3):
        try:
            res = run_bass_kernel_spmd(nc, in_maps, core_ids=list(range(NCORES)))
            break
        except Exception as e:  # transient NRT_EXEC_UNIT_UNRECOVERABLE faults
            last_err = e
    else:
        raise last_err
    out = np.concatenate([res.results[c]["out"] for c in range(NCORES)], axis=1)
    return out.astype(np.float32)


if __name__ == "__main__":
    rng = np.random.default_rng(0)
    buf = rng.random((B, T, P), dtype=np.float32)
    weight = rng.standard_normal((P, QFULL), dtype=np.float32) * np.sqrt(2.0 / P)
    delay_raw = rng.standard_normal((P, QFULL), dtype=np.float32)
    out = kernel(buf=buf, weight=weight, delay_raw=delay_raw)
    print("out", out.shape, out.dtype, float(np.abs(out).max()))
